# revision 9
# baseline (speedup 1.0000x reference)
"""Trainium2 Bass kernel for nn_ContrastiveMoCo (B=256, H=768, K=65536, L=10).

v2 strategy (8 NeuronCores, SPMD, fp8 + DoubleRow):
- Masked logsumexp over all negatives replaces the reference's top_k sort
  (validated: ~7e-5 relative on the loss).
- The [K,H] queue is the data floor: surviving 65280 rows are sharded
  8160/core, padded to 8192 columns, scaled x256 and stored fp8e4m3 in a
  [128, 6, 8192] DoubleRow-friendly layout.  Padded columns contribute
  exactly exp(-SHIFT) each; subtracted in the host combine.
- All matmuls (heads + queue) run fp8 DoubleRow (0.5 cyc/row): weights
  scaled x32 (scale folded into the activation), q operand x8, Wc2 x64.
- The label mask rides as a 10-row DoubleRow matmul pair ([5,2,*]) with
  -240*onehot(labels) x 240*onehot(label_queue) products: -57600 in PSUM
  kills masked entries through the exp.
- DMA instruction count is minimized (the HWDGE + DMA-engine serialization
  dominated the old kernel): one DMA each for x/weights-q/weights-kc/mask,
  8 chunked DMAs for the queue shard, a handful of small constant loads.
- Norm scales via Ln+Exp (one act-table reload); all tanh emitted first.
- Host combines per-core (sumexp, norms, l_pos, cls-CE parts) in f64.
"""

import numpy as np
import ml_dtypes

import concourse.bacc as bacc
import concourse.tile as tile
from concourse import mybir
from concourse.bass_utils import run_bass_kernel_spmd

f32 = mybir.dt.float32
bf16 = mybir.dt.bfloat16
f8 = mybir.dt.float8e4
AF = mybir.ActivationFunctionType
DR = mybir.MatmulPerfMode.DoubleRow
E4 = ml_dtypes.float8_e4m3
BF = ml_dtypes.bfloat16

B, H, K, L = 256, 768, 65536, 10
M_MOM, TEMP, C_RATE = 0.999, 0.07, 0.1
NCORES = 8
HCH = 6                      # H / 128 contraction chunks
KC = (K - B) // NCORES       # 8160 surviving queue cols per core
KCP = 8192                   # padded (512-aligned) per-core cols
NCH = 8                      # queue DMA chunks
JC = KCP // NCH              # 1024 cols per chunk
SHIFT = 16.0                 # fixed logsumexp shift
WS = 32.0                    # weight scale (heads)
QS = 8.0                     # liner_q fp8 operand scale
FS = 256.0                   # feature-queue fp8 scale
CS = 64.0                    # Wc2 fp8 scale
MS = 240.0                   # mask onehot magnitude (fp8e4m3 max normal)
PENBF = 1.0e9                # bf16 mask penalty (extra block)


def build_nc(parts=("heads", "stats", "extra", "cls", "main"), with_bias=False):
    nc = bacc.Bacc()

    x8 = nc.dram_tensor("x8", [128, 2 * HCH, B], f8, kind="ExternalInput")
    w8 = nc.dram_tensor("w8", [128, 5 * HCH, H], f8, kind="ExternalInput")
    wc2 = nc.dram_tensor("wc2", [128, HCH, L], f8, kind="ExternalInput")
    bb = (nc.dram_tensor("bb", [1, 5 * H], bf16, kind="ExternalInput")
          if with_bias else None)
    extl = nc.dram_tensor("extl", [L, B], bf16, kind="ExternalInput")
    ohl = nc.dram_tensor("ohl", [L, B], bf16, kind="ExternalInput")
    e8d = nc.dram_tensor("e8d", [5, 2, B], f8, kind="ExternalInput")
    mq8 = nc.dram_tensor("mq8", [5, 2, KCP], f8, kind="ExternalInput")
    pick = nc.dram_tensor("pick", [128, 2 * L], f32, kind="ExternalInput")
    bc2 = nc.dram_tensor("bc2", [128, L], f32, kind="ExternalInput")
    fq8 = nc.dram_tensor("fq8", [128, HCH, KCP], f8, kind="ExternalInput")
    OUT = nc.dram_tensor("out", [128, 14], f32, kind="ExternalOutput")

    # queue col groups: 5 x 1536 + 1 x 512 = 8192
    GSZ = [1536] * 5 + [512]
    GOF = [0, 1536, 3072, 4608, 6144, 7680]
    NG = 6

    with tile.TileContext(nc) as tc:
        with (
            tc.tile_pool(name="cst", bufs=1) as cp,
            tc.tile_pool(name="fqp", bufs=1) as fp,
            tc.tile_pool(name="scr", bufs=3) as sp,
            tc.tile_pool(name="pb", bufs=2, space="PSUM") as pb,
            tc.tile_pool(name="pst", bufs=2, space="PSUM") as pst,
        ):
            # ---- small DMAs on Pool/SWDGE (keeps HWDGE free for bulk) ----
            mqt = cp.tile([5, 2, KCP], f8, tag="mqt")
            nc.gpsimd.dma_start(mqt[:], mq8[:])
            e8t = cp.tile([5, 2, B], f8, tag="e8t")
            nc.gpsimd.dma_start(e8t[:], e8d[:])
            if with_bias:
                bbt = cp.tile([1, 5 * H], bf16, tag="bbt")
                nc.gpsimd.dma_start(bbt[:], bb[:])
            else:
                bbt = None
            extlt = cp.tile([L, B], bf16, tag="extlt")
            nc.gpsimd.dma_start(extlt[:], extl[:])
            ohlt = cp.tile([L, B], bf16, tag="ohlt")
            nc.gpsimd.dma_start(ohlt[:], ohl[:])
            wc2t = cp.tile([128, HCH, L], f8, tag="wc2t")
            nc.gpsimd.dma_start(wc2t[:], wc2[:])
            pickt = cp.tile([128, 2 * L], f32, tag="pickt")
            nc.gpsimd.dma_start(pickt[:], pick[:])
            bc2t = cp.tile([128, L], f32, tag="bc2t")
            nc.gpsimd.dma_start(bc2t[:], bc2[:])
            # ---- bulk DMAs on SP/HWDGE: q-path first, then queue + k-weights
            xt = cp.tile([128, 2 * HCH, B], f8, tag="xt")
            nc.sync.dma_start(xt[:], x8[:])
            wqt = cp.tile([128, 2 * HCH, H], f8, tag="wqt")      # q1, q2
            nc.sync.dma_start(wqt[:, 0:HCH, :], w8[:, 0:HCH, :])
            nc.sync.dma_start(wqt[:, HCH:2 * HCH, :], w8[:, HCH:2 * HCH, :])
            fqt = [fp.tile([128, HCH, GSZ[g]], f8, tag=f"fq{g}", name=f"fq{g}")
                   for g in range(NG)]
            wkt = cp.tile([128, 3 * HCH, H], f8, tag="wkt")      # k1, c1, k2
            nc.sync.dma_start(wkt[:, 0:2 * HCH, :], w8[:, 12:24, :])
            nc.sync.dma_start(fqt[0][:], fq8[:, :, 0:GSZ[0]])
            nc.sync.dma_start(wkt[:, 2 * HCH:3 * HCH, :], w8[:, 24:30, :])
            for g in range(1, NG):
                nc.sync.dma_start(fqt[g][:], fq8[:, :, GOF[g]:GOF[g] + GSZ[g]])

            # ---- constants ----
            ones_r = cp.tile([1, B], bf16, tag="ones_r")
            nc.vector.memset(ones_r[:], 1.0)
            ones_c = cp.tile([128, 1], bf16, tag="ones_c")
            nc.vector.memset(ones_c[:], 1.0)
            bz = cp.tile([128, 1], f32, tag="bz")
            nc.vector.memset(bz[:], 0.0)
            bsh = cp.tile([128, 1], f32, tag="bsh")
            nc.vector.memset(bsh[:], -SHIFT)

            out_sb = cp.tile([128, 14], f32, tag="out_sb")
            secol = [cp.tile([128, NG], f32, tag=f"secol{it}",
                             name=f"secol{it}") for it in range(2)]

            # ---- heads: fp8 DoubleRow layers ----
            def layer(wt, wbase, rhs_t, rbase, bcol, out_cb):
                """One 768x768 layer: out_cb(g, psum[:, 0:3, :]) per half."""
                for g in range(2):
                    ps = pb.tile([128, HCH, B], f32, tag="pb")
                    for m3 in range(3):
                        m = 3 * g + m3
                        for kp in range(3):
                            nc.tensor.matmul(
                                ps[:, m3, :],
                                wt[:, wbase + 2 * kp:wbase + 2 * kp + 2,
                                   m * 128:(m + 1) * 128],
                                rhs_t[:, rbase + 2 * kp:rbase + 2 * kp + 2, :],
                                start=(kp == 0),
                                stop=(not with_bias and kp == 2), perf_mode=DR)
                        if with_bias:
                            nc.tensor.matmul(
                                ps[:, m3, :],
                                bbt[0:1, bcol * H + m * 128:bcol * H + (m + 1) * 128],
                                ones_r[0:1, :], start=False, stop=True)
                    out_cb(g, ps)

            tq8 = cp.tile([128, HCH, B], f8, tag="tq8")
            tk8 = cp.tile([128, HCH, B], f8, tag="tk8")
            tc8 = cp.tile([128, HCH, B], f8, tag="tc8")
            qbf = cp.tile([128, HCH, B], bf16, tag="qbf")
            kf = cp.tile([128, HCH, B], bf16, tag="kf")

            def tanh_out(t8):
                def cb(g, ps):
                    nc.scalar.activation(t8[:, 3 * g:3 * g + 3, :],
                                         ps[:, 0:3, :],
                                         AF.Tanh, bias=bz[:], scale=1.0 / WS)
                return cb

            def bf_out(obf):
                def cb(g, ps):
                    nc.vector.tensor_scalar_mul(obf[:, 3 * g:3 * g + 3, :],
                                                ps[:, 0:3, :], 1.0 / WS)
                return cb

            def rsqrt_newton(x_ap, shape, tag, iters=3):
                """y ~= x^-1/2 on DVE only (mult/add/recip), Newton iters."""
                r = cp.tile(shape, f32, tag=f"nr_{tag}", name=f"nr_{tag}")
                nc.vector.reciprocal(r[:], x_ap)
                y = cp.tile(shape, f32, tag=f"ny_{tag}", name=f"ny_{tag}")
                nc.vector.tensor_scalar(y[:], r[:], 3.94, 0.0594,
                                        op0=mybir.AluOpType.mult,
                                        op1=mybir.AluOpType.add)
                t = cp.tile(shape, f32, tag=f"nt_{tag}", name=f"nt_{tag}")
                for _ in range(iters):
                    nc.vector.tensor_mul(t[:], y[:], y[:])
                    nc.vector.tensor_mul(t[:], t[:], x_ap)
                    nc.vector.tensor_scalar(t[:], t[:], -0.5, 1.5,
                                            op0=mybir.AluOpType.mult,
                                            op1=mybir.AluOpType.add)
                    nc.vector.tensor_mul(y[:], y[:], t[:])
                return y

            def colsum(src, it):
                ps = pst.tile([128, 1], f32, tag="pst", padded_shape=[128, B])
                for m in range(HCH):
                    nc.tensor.matmul(
                        ps[:], src[:, m, it * 128:(it + 1) * 128],
                        ones_c[:], start=(m == 0), stop=(m == HCH - 1))
                return ps

            qbf8 = cp.tile([128, HCH, B], f8, tag="qbf8")
            sq_q = cp.tile([128, HCH, B], bf16, tag="sq_q")
            sq_k = cp.tile([128, HCH, B], bf16, tag="sq_k")
            pkm = cp.tile([128, HCH, B], bf16, tag="pkm")
            knbf = cp.tile([128, HCH, B], bf16, tag="knbf")
            bc16 = cp.tile([128, B], bf16, tag="bc16")
            sdev, sxs = [], []

            # ---- q head, then k1/c1 tanh early (fills idle Act window),
            # ---- then q stats (critical path to the main loop) ----
            if "heads" in parts:
                layer(wqt, 0, xt, 0, 0, tanh_out(tq8))        # q1
                layer(wqt, HCH, tq8, 0, 1, bf_out(qbf))       # q2
                layer(wkt, 0, xt, HCH, 2, tanh_out(tk8))      # k1
                layer(wkt, HCH, xt, 0, 3, tanh_out(tc8))      # c1
            if "stats" in parts:
                nc.scalar.activation(qbf8[:, 0:3, :], qbf[:, 0:3, :],
                                     AF.Copy, scale=QS)
                nc.vector.tensor_scalar_mul(qbf8[:, 3:6, :], qbf[:, 3:6, :], QS)
                for g in range(2):
                    sl = slice(3 * g, 3 * g + 3)
                    nc.vector.tensor_mul(sq_q[:, sl, :], qbf[:, sl, :],
                                         qbf[:, sl, :])
                ssq2 = cp.tile([128, 2], f32, tag="ssq2")
                for it in range(2):
                    ps_ssq = colsum(sq_q, it)
                    nc.vector.tensor_copy(out_sb[:, 4 + it:5 + it], ps_ssq[:])
                    nc.vector.tensor_copy(ssq2[:, it:it + 1], ps_ssq[:])
                y2 = rsqrt_newton(ssq2[:], [128, 2], "s", iters=2)
                sdev2 = cp.tile([128, 2], f32, tag="sdev2")
                nc.vector.tensor_scalar_mul(sdev2[:], y2[:],
                                            float(1.0 / (TEMP * QS * FS)))
                sx2 = cp.tile([128, 2], f32, tag="sx2")
                nc.vector.tensor_scalar_mul(sx2[:], y2[:], float(1.0 / TEMP))
                sdev = [sdev2[:, 0:1], sdev2[:, 1:2]]
                sxs = [sx2[:, 0:1], sx2[:, 1:2]]

            # ---- main-loop group emitter ----
            main_on = "main" in parts

            def main_group(g):
                npc = GSZ[g] // 512
                for it in range(2):
                    mp = pb.tile([128, HCH, B], f32, tag="pb")
                    for p in range(npc):
                        for kp in range(3):
                            nc.tensor.matmul(
                                mp[:, 2 * p:2 * p + 2, :],
                                qbf8[:, 2 * kp:2 * kp + 2,
                                     it * 128:(it + 1) * 128],
                                fqt[g][:, 2 * kp:2 * kp + 2,
                                       p * 512:(p + 1) * 512],
                                start=(kp == 0), stop=False, perf_mode=DR)
                        nc.tensor.matmul(
                            mp[:, 2 * p:2 * p + 2, :],
                            e8t[:, :, it * 128:(it + 1) * 128],
                            mqt[:, :, GOF[g] + p * 512:GOF[g] + (p + 1) * 512],
                            start=False, stop=True, perf_mode=DR)
                    scr = sp.tile([128, HCH, B], bf16, tag="scr")
                    nc.scalar.activation(scr[:, 0:2 * npc, :],
                                         mp[:, 0:2 * npc, :], AF.Exp,
                                         bias=bsh[:], scale=sdev[it],
                                         accum_out=secol[it][:, g:g + 1])

            if main_on:
                main_group(0)

            if "heads" in parts:
                layer(wkt, 2 * HCH, tk8, 0, 4, bf_out(kf))    # k2

            if main_on:
                for g in (1, 2):
                    main_group(g)

            # ---- k stats: norms, l_pos parts, normalized keys ----
            if "stats" in parts:
                for g in range(2):
                    sl = slice(3 * g, 3 * g + 3)
                    nc.vector.tensor_mul(sq_k[:, sl, :], kf[:, sl, :],
                                         kf[:, sl, :])
                    nc.vector.tensor_mul(pkm[:, sl, :], qbf[:, sl, :],
                                         kf[:, sl, :])
                for it in range(2):
                    ps_ssk = colsum(sq_k, it)
                    nc.vector.tensor_copy(out_sb[:, 6 + it:7 + it], ps_ssk[:])
                    ps_pk = colsum(pkm, it)
                    nc.vector.tensor_copy(out_sb[:, 8 + it:9 + it], ps_pk[:])
                ps_kr = pst.tile([1, B], f32, tag="pst", padded_shape=[128, B])
                for m in range(HCH):
                    nc.tensor.matmul(ps_kr[:], ones_c[:], sq_k[:, m, :],
                                     start=(m == 0), stop=(m == HCH - 1))
                yk = rsqrt_newton(ps_kr[:], [1, B], "kr")
                invk = cp.tile([1, B], bf16, tag="invk")
                nc.vector.tensor_copy(invk[:], yk[:])
                ps_bc = pst.tile([128, B], f32, tag="pst", padded_shape=[128, B])
                nc.tensor.matmul(ps_bc[:], ones_r[0:1, 0:128], invk[:],
                                 start=True, stop=True)
                nc.vector.tensor_copy(bc16[:], ps_bc[:])
                for m in range(HCH):
                    nc.vector.tensor_mul(knbf[:, m, :], kf[:, m, :], bc16[:])

            if main_on:
                for g in (3, 4):
                    main_group(g)

            # ---- extra block: 256 update-key negatives (bf16) ----
            if "extra" in parts:
                for it in range(2):
                    ps_x = pst.tile([128, B], f32, tag="pst",
                                    padded_shape=[128, B])
                    for m in range(HCH):
                        nc.tensor.matmul(ps_x[:],
                                         qbf[:, m, it * 128:(it + 1) * 128],
                                         knbf[:, m, :], start=(m == 0),
                                         stop=False)
                    nc.tensor.matmul(ps_x[:], extlt[:, it * 128:(it + 1) * 128],
                                     ohlt[:], start=False, stop=True)
                    scx = sp.tile([128, HCH, B], bf16, tag="scr")
                    nc.scalar.activation(scx[:, 0, :], ps_x[:], AF.Exp,
                                         bias=bsh[:], scale=sxs[it],
                                         accum_out=out_sb[:, 2 + it:3 + it])

            # ---- classifier CE parts ----
            if "cls" in parts:
                for it in range(2):
                    ps_c = pst.tile([128, L], f32, tag="pst",
                                    padded_shape=[128, B])
                    for kp in range(3):
                        nc.tensor.matmul(
                            ps_c[:],
                            tc8[:, 2 * kp:2 * kp + 2, it * 128:(it + 1) * 128],
                            wc2t[:, 2 * kp:2 * kp + 2, :],
                            start=(kp == 0), stop=(kp == 2), perf_mode=DR)
                    lg = cp.tile([128, L], f32, tag=f"lg{it}", name=f"lg{it}")
                    nc.vector.tensor_scalar_mul(lg[:], ps_c[:], 1.0 / CS)
                    lg2 = cp.tile([128, L], f32, tag=f"lg2{it}", name=f"lg2{it}")
                    nc.vector.tensor_add(lg2[:], lg[:], bc2t[:])
                    esc = cp.tile([128, L], f32, tag=f"esc{it}", name=f"esc{it}")
                    nc.scalar.activation(esc[:], lg2[:], AF.Exp, bias=bz[:],
                                         accum_out=out_sb[:, 10 + it:11 + it])
                    pkc = cp.tile([128, L], f32, tag=f"pkc{it}", name=f"pkc{it}")
                    nc.vector.tensor_mul(pkc[:], lg2[:],
                                         pickt[:, it * L:(it + 1) * L])
                    nc.vector.reduce_sum(out_sb[:, 12 + it:13 + it], pkc[:],
                                         axis=mybir.AxisListType.X)

            if main_on:
                main_group(5)
                for it in range(2):
                    nc.vector.reduce_sum(out_sb[:, it:it + 1], secol[it][:],
                                         axis=mybir.AxisListType.X)

            nc.sync.dma_start(OUT[:], out_sb[:])
    nc.finalize()
    return nc


_NC_CACHE = {}


def _get_nc(with_bias=False):
    if with_bias not in _NC_CACHE:
        _NC_CACHE[with_bias] = build_nc(with_bias=with_bias)
    return _NC_CACHE[with_bias]


def _chunked(M, scale, dt):
    """[H, N] -> [128, HCH, N] h-chunked layout."""
    Hd, N = M.shape
    return np.ascontiguousarray(
        (M * scale).reshape(HCH, 128, N).transpose(1, 0, 2)).astype(dt)


def _prepare(pooled_q, pooled_p, labels, label_queue, feature_queue,
             Wq1, bq1, Wq2, bq2, Wk1, bk1, Wk2, bk2,
             Wc1, bc1, Wc2, bc2, ptr):
    pooled_q = np.asarray(pooled_q, np.float32)
    pooled_p = np.asarray(pooled_p, np.float32)
    labels = np.asarray(labels)
    label_queue = np.asarray(label_queue)
    feature_queue = np.asarray(feature_queue, np.float32)
    ptr_i = int(np.asarray(ptr))

    Wk1n = (np.float32(M_MOM) * np.asarray(Wk1, np.float32)
            + np.float32(1 - M_MOM) * np.asarray(Wq1, np.float32))
    Wk2n = (np.float32(M_MOM) * np.asarray(Wk2, np.float32)
            + np.float32(1 - M_MOM) * np.asarray(Wq2, np.float32))
    bk1n = (np.float32(M_MOM) * np.asarray(bk1, np.float32)
            + np.float32(1 - M_MOM) * np.asarray(bq1, np.float32))
    bk2n = (np.float32(M_MOM) * np.asarray(bk2, np.float32)
            + np.float32(1 - M_MOM) * np.asarray(bq2, np.float32))

    idx = (ptr_i + np.arange(B)) % K
    keep_mask = np.ones(K, bool)
    keep_mask[idx] = False
    keep = np.flatnonzero(keep_mask)          # 65280 surviving queue rows
    lab = labels.astype(np.int64)

    # weights: [q1, q2, k1, k2, c1] x32, h-chunked, fp8
    w8 = np.concatenate(
        [_chunked(np.asarray(W, np.float32), WS, E4)
         for W in (Wq1, Wq2, Wk1n, Wc1, Wk2n)], axis=1)
    x8 = np.concatenate([_chunked(pooled_q.T, 1.0, E4),
                         _chunked(pooled_p.T, 1.0, E4)], axis=1)
    bb = np.concatenate(
        [WS * np.asarray(b, np.float32)
         for b in (bq1, bq2, bk1n, bc1, bk2n)])[None, :].astype(BF)

    oh = (lab[None, :] == np.arange(L)[:, None])          # [L, B]
    e8d = np.zeros((5, 2, B), np.float32)
    e8d[lab % 5, lab // 5, np.arange(B)] = -MS

    pick = np.zeros((128, 2 * L), np.float32)
    for it in range(2):
        pick[np.arange(128), it * L + lab[it * 128:(it + 1) * 128]] = 1.0

    with_bias = bool(np.any(bb.astype(np.float32)))
    common = {
        "x8": x8, "w8": w8,
        "wc2": _chunked(np.asarray(Wc2, np.float32), CS, E4),
        "extl": np.ascontiguousarray((-PENBF * oh).astype(BF)),
        "ohl": np.ascontiguousarray(oh.astype(BF)),
        "e8d": e8d.astype(E4),
        "pick": pick,
        "bc2": np.ascontiguousarray(
            np.broadcast_to(np.asarray(bc2, np.float32)[None, :], (128, L))),
    }
    lq_keep = label_queue[keep].astype(np.int64)
    in_maps = []
    for c in range(NCORES):
        sl = keep[c * KC:(c + 1) * KC]
        fqT = np.zeros((H, KCP), np.float32)
        fqT[:, :KC] = feature_queue[sl].T * FS
        lqs = lq_keep[c * KC:(c + 1) * KC]
        mq8 = np.zeros((5, 2, KCP), np.float32)
        mq8[lqs % 5, lqs // 5, np.arange(KC)] = MS
        m = dict(common)
        m["fq8"] = np.ascontiguousarray(
            fqT.reshape(HCH, 128, KCP).transpose(1, 0, 2)).astype(E4)
        m["mq8"] = mq8.astype(E4)
        if with_bias:
            m["bb"] = bb
        in_maps.append(m)
    return in_maps, with_bias, idx, labels, label_queue


def _combine(results, idx, labels, label_queue):
    outs = [r["out"].astype(np.float64) for r in results]

    def col(o, base):  # columns (base, base+1) -> [256]
        return np.concatenate([o[:, base], o[:, base + 1]])

    pad_leak = NCORES * (KCP - KC) * np.exp(-SHIFT)
    se_main = sum(col(o, 0) for o in outs) - pad_leak
    o0 = outs[0]
    se_x = col(o0, 2)
    ssq = col(o0, 4)
    ssk = col(o0, 6)
    rawlpos = col(o0, 8)
    se_cls = col(o0, 10)
    pick = col(o0, 12)

    lpos_t = rawlpos / (np.sqrt(ssq * ssk) * TEMP)
    total = se_main + se_x + np.exp(lpos_t - SHIFT)
    S = np.log(total) + SHIFT
    loss_con = np.mean(S - lpos_t)
    loss_cls = np.mean(np.log(se_cls) - pick)

    lab = np.asarray(labels).astype(np.int64)
    lq_new = np.asarray(label_queue).copy()
    lq_new[idx] = np.asarray(labels).astype(lq_new.dtype)
    hist = np.bincount(lq_new.astype(np.int64), minlength=L)
    neg_min = K - hist[lab].max()

    loss = C_RATE * loss_con + (1 - C_RATE) * loss_cls if neg_min > 0 else loss_cls
    return np.float32(loss)


def kernel(**inputs):
    in_maps, with_bias, idx, labels, label_queue = _prepare(**inputs)
    nc = _get_nc(with_bias)
    res = run_bass_kernel_spmd(nc, in_maps, list(range(NCORES)))
    return _combine(res.results, idx, labels, label_queue)


def run_traced(inputs):
    """Dev-only: run once with NTFF tracing; returns (exec_time_ns, loss)."""
    in_maps, with_bias, idx, labels, label_queue = _prepare(**inputs)
    nc = _get_nc(with_bias)
    res = run_bass_kernel_spmd(nc, in_maps, list(range(NCORES)), trace=True)
    loss = _combine(res.results, idx, labels, label_queue)
    return res.exec_time_ns, loss
